# revision 25
# baseline (speedup 1.0000x reference)
import json
import math

import numpy as np

# Problem constants (nn_Attention_83502754169400): hardcoded per contract.
B, S, D, H = 2, 2048, 2048, 16
HD = D // H          # 128
NCORES = 8
HL = H // NCORES     # heads per core = 2
DL = HL * HD         # per-core projected width = 256
BS = B * S           # 4096
EPS = 1e-5
ISQ = 1.0 / math.sqrt(HD)

NQT = BS // 128      # 32 q-tiles (seq tiles across both batches)
SQT = S // 128       # 16 seq tiles per batch
ICH = 512            # i-chunk width in attention
NIC = S // ICH       # 4 i-chunks per batch
NKT = D // 128       # 16 contraction tiles over D


# ---------------------------------------------------------------------------
# Workaround: this walrus build rejects >1 semaphore wait per instruction
# ("Too many sync wait commands").  Split extra waits onto preceding Drains.
# ---------------------------------------------------------------------------

def _split_waits_json(bir_json, max_waits=1):
    j = json.loads(bir_json)
    for fn in j.get("functions", []):
        for bb in fn.get("blocks", []):
            insts = bb.get("instructions", [])
            out = []
            for inst in insts:
                si = inst.get("sync_info") or {}
                waits = si.get("on_wait") or []
                if len(waits) > max_waits:
                    chunks = [waits[i:i + max_waits]
                              for i in range(0, len(waits), max_waits)]
                    for k, ch in enumerate(chunks[:-1]):
                        out.append({
                            "debug": inst.get("debug"),
                            "engine": inst["engine"],
                            "ins": [],
                            "is_reset_sema": False,
                            "name": "%s__ws%d" % (inst["name"], k),
                            "opcode": "Drain",
                            "outs": [],
                            "sync_info": {"on_update": [], "on_wait": ch},
                        })
                    si["on_wait"] = chunks[-1]
                    inst["sync_info"] = si
                out.append(inst)
            bb["instructions"] = out
    return json.dumps(j).encode()


_WAIT_SPLIT_INSTALLED = False


def _install_wait_split():
    global _WAIT_SPLIT_INSTALLED
    if _WAIT_SPLIT_INSTALLED:
        return
    import concourse.bass2jax as b2j
    import concourse.bass_utils as bu
    orig = bu.compile_bir_kernel

    def patched(bir_json, tmpdir, neff_name="file.neff"):
        return orig(_split_waits_json(bir_json), tmpdir, neff_name)

    b2j.compile_bir_kernel = patched
    bu.compile_bir_kernel = patched
    _WAIT_SPLIT_INSTALLED = True


# ---------------------------------------------------------------------------
# Bass kernel builder (SPMD; per-core data comes from per-core input shards).
#
# Per core c (owning heads 2c, 2c+1):
#   phase 1: x -> xT (PE transpose), QKV proj (bf16), LN+RoPE on Q/K,
#            PE-transpose Qn/Kn -> QT/KT [hd, seq]; V kept natural [seq, hd].
#   phase 2: per (b, h, 512-wide i-chunk): S^T = KT_j.T @ QT_i for causal
#            j-tiles, exp on ACT (scale 1/sqrt(hd)), diagonal causal mask via
#            gpsimd affine_select, O^T += V_j.T @ expS; denominators via DVE
#            accumulate + ones-matmul; normalize O^T with K=1 bcast matmul.
#   phase 3: O^T -> DRAM, AllGather over 8 cores (one per batch),
#            out[:, c*256:(c+1)*256] = O_full @ wo[:, c-slice].
# ---------------------------------------------------------------------------

_NC_CACHE = {}


def _build_nc(use_w, use_b):
    import concourse.bass as bass
    import concourse.mybir as mybir
    from concourse import tile
    from contextlib import ExitStack

    f32 = mybir.dt.float32
    bf16 = mybir.dt.bfloat16
    fp8 = mybir.dt.float8e4
    AF = mybir.ActivationFunctionType
    ALU = mybir.AluOpType

    nc = bass.Bass(num_devices=NCORES)

    x = nc.dram_tensor("x", [BS, D], f32, kind="ExternalInput")
    wq = nc.dram_tensor("wq", [D, DL], f32, kind="ExternalInput")
    wk = nc.dram_tensor("wk", [D, DL], f32, kind="ExternalInput")
    wv = nc.dram_tensor("wv", [D, DL], f32, kind="ExternalInput")
    wo = nc.dram_tensor("wo", [D, DL], f32, kind="ExternalInput")
    fcos = nc.dram_tensor("fcos", [S, HD // 2], f32, kind="ExternalInput")
    fsin = nc.dram_tensor("fsin", [S, HD // 2], f32, kind="ExternalInput")
    # packed LN params: [we_q, wo_q, we_k, wo_k, be_q, bo_q, be_k, bo_k] x 64
    lnwb = nc.dram_tensor("lnwb", [1, 512], f32, kind="ExternalInput")
    eye = nc.dram_tensor("eye", [128, 128], f32, kind="ExternalInput")
    ones = nc.dram_tensor("ones", [128, 128], f32, kind="ExternalInput")
    out = nc.dram_tensor("out", [DL, BS], f32, kind="ExternalOutput")

    def bc2(ap, n):
        # [128, F] -> [128, n, F] free-dim broadcast
        f = ap.shape[-1]
        return ap.rearrange("p (o f) -> p o f", o=1).broadcast_to([128, n, f])

    with tile.TileContext(nc) as tc, ExitStack() as ctx:
        E = ctx.enter_context
        const_pool = E(tc.tile_pool(name="const", bufs=1))
        wpool = E(tc.tile_pool(name="wpool", bufs=1))
        resident = E(tc.tile_pool(name="resident", bufs=1))
        dram_pool = E(tc.tile_pool(name="dram", bufs=1, space="DRAM"))

        # collective bounce buffers (two AllGathers per batch, by i-range)
        SH = S // 2
        ag_in = [[dram_pool.tile([DL, SH], bf16, name="ag_in%d_%d" % (b, hf))
                  for hf in range(2)] for b in range(B)]
        ag_out = [[dram_pool.tile([D, SH], bf16, addr_space="Shared",
                                  name="ag_out%d_%d" % (b, hf))
                   for hf in range(2)] for b in range(B)]

        # ---- constants (persistent bf16 + transient f32 staging) --------
        eye_b = const_pool.tile([128, 128], bf16, tag="eye_b")
        ones_f = const_pool.tile([1, 128], f32, tag="ones_f")
        nc.sync.dma_start(ones_f[:], ones[0:1, :])
        ones_sq_b = const_pool.tile([128, 128], bf16, tag="ones_sq_b")
        eps_t = const_pool.tile([128, 1], f32, tag="eps_t")
        nc.vector.memset(eps_t[:], EPS)
        dmask = const_pool.tile([128, 4, ICH], bf16, tag="dmask")
        nc.vector.memset(dmask[:], 1.0)
        for kk in range(4):
            nc.gpsimd.affine_select(
                out=dmask[:, kk, :], in_=dmask[:, kk, :],
                pattern=[[1, ICH]], base=-128 * kk, channel_multiplier=-1,
                compare_op=ALU.is_ge, fill=0.0)
        cos_b = const_pool.tile([128, SQT, HD // 2], bf16, tag="cos_b")
        sin_b = const_pool.tile([128, SQT, HD // 2], bf16, tag="sin_b")
        cos_sb = None
        sin_sb = None

        with tc.tile_pool(name="startup", bufs=2) as startup:
            eye_f = startup.tile([128, 128], f32, tag="sustage")
            nc.sync.dma_start(eye_f[:], eye[:, :])
            nc.vector.tensor_copy(eye_b[:], eye_f[:])
            ones_sq = startup.tile([128, 128], f32, tag="sustage")
            nc.sync.dma_start(ones_sq[:], ones[:, :])
            nc.vector.tensor_copy(ones_sq_b[:], ones_sq[:])
            if use_w or use_b:
                cos_sb = const_pool.tile([128, SQT, HD // 2], f32,
                                         tag="cos_sb")
                sin_sb = const_pool.tile([128, SQT, HD // 2], f32,
                                         tag="sin_sb")
                nc.sync.dma_start(
                    cos_sb[:], fcos.rearrange("(t p) f -> p t f", p=128))
                nc.sync.dma_start(
                    sin_sb[:], fsin.rearrange("(t p) f -> p t f", p=128))
                nc.vector.tensor_copy(cos_b[:], cos_sb[:])
                nc.vector.tensor_copy(sin_b[:], sin_sb[:])
            else:
                cstage = startup.tile([128, SQT, HD // 2], f32, tag="fstage")
                nc.sync.dma_start(
                    cstage[:], fcos.rearrange("(t p) f -> p t f", p=128))
                nc.vector.tensor_copy(cos_b[:], cstage[:])
                sstage = startup.tile([128, SQT, HD // 2], f32, tag="fstage")
                nc.sync.dma_start(
                    sstage[:], fsin.rearrange("(t p) f -> p t f", p=128))
                nc.vector.tensor_copy(sin_b[:], sstage[:])

        # LN param broadcast via K=1 matmul: [1,512] -> [128,512]
        if use_w or use_b:
            lnwb_sb = const_pool.tile([1, 512], f32, tag="lnwb_sb")
            nc.sync.dma_start(lnwb_sb[:], lnwb[:, :])
            ln_bc = const_pool.tile([128, 512], f32, tag="ln_bc")
            with tc.tile_pool(name="bc_ps", bufs=1, space="PSUM") as bc_ps:
                lnp = bc_ps.tile([128, 512], f32, tag="lnp")
                nc.tensor.matmul(lnp[:], ones_f[:, :], lnwb_sb[:, :])
                nc.scalar.copy(ln_bc[:], lnp[:])
            w_e = [ln_bc[:, 0:64], ln_bc[:, 128:192]]       # q, k
            w_o = [ln_bc[:, 64:128], ln_bc[:, 192:256]]
            b_e = [ln_bc[:, 256:320], ln_bc[:, 384:448]]
            b_o = [ln_bc[:, 320:384], ln_bc[:, 448:512]]

        # rope coefficients per tensor t in {q,k}:
        #   A=c*we, Bc=s*wo, C=s*we, Dc=c*wo  (plain c/s when w==1)
        if use_w:
            ropeAB = const_pool.tile([128, 2, 4, SQT, HD // 2], bf16,
                                     tag="ropeAB")
            for t in range(2):
                for st in range(SQT):
                    cs = cos_sb[:, st, :]
                    sn = sin_sb[:, st, :]
                    nc.vector.tensor_mul(ropeAB[:, t, 0, st, :], cs, w_e[t])
                    nc.vector.tensor_mul(ropeAB[:, t, 1, st, :], sn, w_o[t])
                    nc.vector.tensor_mul(ropeAB[:, t, 2, st, :], sn, w_e[t])
                    nc.vector.tensor_mul(ropeAB[:, t, 3, st, :], cs, w_o[t])

            def rope_coef4(st):
                def g(i):
                    return ropeAB[:, :, i, st, :].rearrange(
                        "p t (o f) -> p t o f", o=1).broadcast_to(
                        [128, 2, HL, HD // 2])
                return (g(0), g(1), g(2), g(3))
        else:
            def rope_coef4(st):
                def g(ap):
                    return ap.rearrange("p (o f) -> p o f", o=1).rearrange(
                        "p o (u f) -> p o u f", u=1).broadcast_to(
                        [128, 2, HL, HD // 2])
                c = g(cos_b[:, st, :])
                sn = g(sin_b[:, st, :])
                return (c, sn, sn, c)

        if use_b:
            def rope_coefEF4(st):
                def g(i):
                    return ropeEF[:, :, i, st, :].rearrange(
                        "p t (o f) -> p t o f", o=1).broadcast_to(
                        [128, 2, HL, HD // 2])
                return (g(0), g(1))
            ropeEF = const_pool.tile([128, 2, 2, SQT, HD // 2], bf16,
                                     tag="ropeEF")
            tmpEF = const_pool.tile([128, HD // 2], f32, tag="tmpEF")
            for t in range(2):
                for st in range(SQT):
                    cs = cos_sb[:, st, :]
                    sn = sin_sb[:, st, :]
                    # E = be*c - bo*s ; F = be*s + bo*c
                    nc.vector.tensor_mul(ropeEF[:, t, 0, st, :], cs, b_e[t])
                    nc.vector.tensor_mul(tmpEF[:], sn, b_o[t])
                    nc.vector.tensor_sub(ropeEF[:, t, 0, st, :],
                                         ropeEF[:, t, 0, st, :], tmpEF[:])
                    nc.vector.tensor_mul(ropeEF[:, t, 1, st, :], sn, b_e[t])
                    nc.vector.tensor_mul(tmpEF[:], cs, b_o[t])
                    nc.vector.tensor_add(ropeEF[:, t, 1, st, :],
                                         ropeEF[:, t, 1, st, :], tmpEF[:])

        # weights -> SBUF bf16 (chunked f32 staging).  Q|K fused (N=512).
        wqk_sb = wpool.tile([128, NKT, 2 * DL], bf16, tag="wqk_sb")
        w_sb = {"wv": wpool.tile([128, NKT, DL], bf16, tag="wb_wv",
                                 name="wb_wv"),
                "wo": wpool.tile([128, NKT, DL], bf16, tag="wb_wo",
                                 name="wb_wo")}
        with tc.tile_pool(name="wstage", bufs=2) as wstage_pool:
            for g in range(2):
                ks = slice(g * (NKT // 2), (g + 1) * (NKT // 2))
                kr = slice(g * (NKT // 2) * 128, (g + 1) * (NKT // 2) * 128)
                for name, t, dst in (("wq", wq, wqk_sb[:, ks, 0:DL]),
                                     ("wk", wk, wqk_sb[:, ks, DL:2 * DL]),
                                     ("wv", wv, w_sb["wv"][:, ks, :]),
                                     ("wo", wo, w_sb["wo"][:, ks, :])):
                    stg = wstage_pool.tile([128, NKT // 2, DL], f32,
                                           tag="wstg", name="wstg")
                    nc.sync.dma_start(
                        stg[:],
                        t[kr, :].rearrange("(k p) n -> p k n", p=128))
                    nc.vector.tensor_copy(dst, stg[:])

        # resident activation tensors: QTr/KTr [128, (h, s)] per batch,
        # V natural [128, (jt, h, d)] per batch
        QTr = [resident.tile([128, HL, S], bf16, tag="QTr%d" % b,
                             name="QTr%d" % b) for b in range(B)]
        KTr = [resident.tile([128, HL, S], bf16, tag="KTr%d" % b,
                             name="KTr%d" % b) for b in range(B)]
        Vn = [resident.tile([128, SQT, DL], bf16, tag="Vn%d" % b,
                            name="Vn%d" % b) for b in range(B)]

        # ---- phase 1: xT, QKV, LN+RoPE, transposes -----------------------
        with tc.tile_pool(name="ph1_tp_ps", bufs=1, space="PSUM") as tp_ps, \
             tc.tile_pool(name="ph1_qk_ps", bufs=2, space="PSUM") as qk_ps, \
             tc.tile_pool(name="ph1_v_ps", bufs=2, space="PSUM") as v_ps, \
             tc.tile_pool(name="ph1", bufs=3) as ph1, \
             tc.tile_pool(name="ph1s", bufs=6) as ph1s:
            pending = []

            def flush_tpq(n):
                while len(pending) > n:
                    qn_, b_, st_ = pending.pop(0)
                    for t_, dst_ in ((0, QTr), (1, KTr)):
                        tpq = tp_ps.tile([128, DL], bf16, tag="tpq",
                                         name="tpq", bufs=2)
                        for h in range(HL):
                            nc.tensor.transpose(
                                tpq[:, h * HD:(h + 1) * HD],
                                qn_[:, (t_ * HL + h) * HD:
                                    (t_ * HL + h + 1) * HD],
                                eye_b[:])
                        nc.scalar.copy(
                            dst_[b_][:, :, st_ * 128:(st_ + 1) * 128],
                            tpq[:].rearrange("p (h f) -> p h f", h=HL))

            for qt in range(NQT):
                flush_tpq(1)
                b, st = qt // SQT, qt % SQT
                x_t = ph1.tile([128, D], f32, tag="x_t")
                nc.sync.dma_start(x_t[:], x[qt * 128:(qt + 1) * 128, :])
                xb = ph1.tile([128, D], bf16, tag="xb")
                nc.vector.tensor_copy(xb[:], x_t[:])
                xT = ph1.tile([128, D], bf16, tag="xT")
                tpx = tp_ps.tile([128, D], bf16, tag="tpx")
                for k in range(NKT):
                    nc.tensor.transpose(
                        tpx[:, k * 128:(k + 1) * 128],
                        xb[:, k * 128:(k + 1) * 128], eye_b[:])
                nc.scalar.copy(xT[:], tpx[:])

                qk = qk_ps.tile([128, 2 * DL], f32, tag="qk")
                vv = v_ps.tile([128, DL], f32, tag="vv")
                for k in range(NKT):
                    nc.tensor.matmul(qk[:], xT[:, k * 128:(k + 1) * 128],
                                     wqk_sb[:, k, :],
                                     start=(k == 0), stop=(k == NKT - 1))
                for k in range(NKT):
                    nc.tensor.matmul(vv[:], xT[:, k * 128:(k + 1) * 128],
                                     w_sb["wv"][:, k, :],
                                     start=(k == 0), stop=(k == NKT - 1))
                nc.scalar.copy(Vn[b][:, st, :], vv[:])

                # LN on Q and K together: phs [128, (t, h, d)]
                phs = ph1s.tile([128, 2 * DL], f32, tag="phs")
                nc.vector.tensor_copy(phs[:], qk[:])
                s1 = ph1s.tile([128, 4], f32, tag="s1")
                nc.vector.reduce_sum(
                    s1[:], phs[:].rearrange("p (g d) -> p g d", g=4),
                    axis=mybir.AxisListType.X)
                sq = ph1s.tile([128, 2 * DL], f32, tag="sq")
                nc.vector.tensor_mul(sq[:], phs[:], phs[:])
                s2 = ph1s.tile([128, 4], f32, tag="s2")
                nc.vector.reduce_sum(
                    s2[:], sq[:].rearrange("p (g d) -> p g d", g=4),
                    axis=mybir.AxisListType.X)
                mu = ph1s.tile([128, 4], f32, tag="mu")
                nc.vector.tensor_scalar_mul(mu[:], s1[:], 1.0 / HD)
                musq = ph1s.tile([128, 4], f32, tag="musq")
                nc.vector.tensor_mul(musq[:], mu[:], mu[:])
                var = ph1s.tile([128, 4], f32, tag="var")
                nc.vector.tensor_scalar(
                    out=var[:], in0=s2[:], scalar1=1.0 / HD,
                    scalar2=None, op0=ALU.mult)
                nc.vector.tensor_sub(var[:], var[:], musq[:])
                rstd = ph1s.tile([128, 4], f32, tag="rstd")
                nc.scalar.activation(rstd[:], var[:], AF.Ln, bias=eps_t[:])
                nc.scalar.activation(rstd[:], rstd[:], AF.Exp, scale=-0.5)
                nmr = ph1s.tile([128, 4], f32, tag="nmr")
                nc.vector.tensor_scalar_mul(nmr[:], mu[:], -1.0)
                nc.vector.tensor_mul(nmr[:], nmr[:], rstd[:])
                lnq = ph1s.tile([128, 2 * DL], f32, tag="lnq")
                for g in range(4):
                    nc.scalar.activation(
                        lnq[:, g * HD:(g + 1) * HD],
                        phs[:, g * HD:(g + 1) * HD], AF.Identity,
                        bias=nmr[:, g:g + 1], scale=rstd[:, g:g + 1])
                # rope for Q and K, both heads, in one 6-op set
                lq = lnq[:].rearrange("p (t h e f) -> p t h e f", t=2, h=HL,
                                      e=2)
                e_, o_ = lq[:, :, :, 0, :], lq[:, :, :, 1, :]
                qn = ph1s.tile([128, 2 * DL], bf16, tag="qn")
                qv = qn[:].rearrange("p (t h e f) -> p t h e f", t=2, h=HL,
                                     e=2)
                oe, oo = qv[:, :, :, 0, :], qv[:, :, :, 1, :]
                A4, B4, C4, D4 = rope_coef4(st)
                m1 = ph1s.tile([128, DL], f32, tag="m1")
                m1v = m1[:].rearrange("p (t h f) -> p t h f", t=2, h=HL)
                m2 = ph1s.tile([128, DL], f32, tag="m2")
                m2v = m2[:].rearrange("p (t h f) -> p t h f", t=2, h=HL)
                nc.vector.tensor_mul(m1v, e_, A4)
                nc.vector.tensor_mul(m2v, o_, B4)
                if use_b:
                    E4, F4 = rope_coefEF4(st)
                    nc.vector.tensor_sub(m1v, m1v, m2v)
                    nc.vector.tensor_add(oe, m1v, E4)
                    nc.vector.tensor_mul(m2v, o_, D4)
                    nc.vector.tensor_mul(m1v, e_, C4)
                    nc.vector.tensor_add(m1v, m1v, m2v)
                    nc.vector.tensor_add(oo, m1v, F4)
                else:
                    nc.vector.tensor_sub(oe, m1v, m2v)
                    nc.vector.tensor_mul(m1v, e_, C4)
                    nc.vector.tensor_mul(m2v, o_, D4)
                    nc.vector.tensor_add(oo, m1v, m2v)
                pending.append((qn, b, st))
            flush_tpq(0)

        # ---- phase 2 + 3: attention, AllGather, wo -----------------------
        with tc.tile_pool(name="ph2_s_ps", bufs=2, space="PSUM") as s_ps, \
             tc.tile_pool(name="ph2_o_ps", bufs=1, space="PSUM") as o_ps, \
             tc.tile_pool(name="ph2_d_ps", bufs=1, space="PSUM") as d_ps, \
             tc.tile_pool(name="ph2", bufs=4) as ph2, \
             tc.tile_pool(name="ph3", bufs=2) as ph3, \
             tc.tile_pool(name="ph3s", bufs=3) as ph3s, \
             tc.tile_pool(name="ph3_ps", bufs=1, space="PSUM") as ph3_ps, \
             nc.allow_low_precision("softmax stats in bf16"):

            def attn_chunk(b, h, ic):
                njt = 4 * (ic + 1)
                ot = o_ps.tile([128, ICH], f32, tag="ot", name="ot")
                acc = ph2.tile([128, ICH], bf16, tag="acc", name="acc")
                qslice = QTr[b][:, h, ic * ICH:(ic + 1) * ICH]
                npr = njt // 2

                def s_pair(pr):
                    sp = s_ps.tile([128, 2 * ICH], f32, tag="sp", name="sp")
                    es = ph2.tile([128, 2 * ICH], bf16, tag="es", name="es")
                    for d2 in range(2):
                        jt = 2 * pr + d2
                        nc.tensor.matmul(
                            sp[:, d2 * ICH:(d2 + 1) * ICH],
                            KTr[b][:, h, jt * 128:(jt + 1) * 128],
                            qslice)
                    nc.scalar.activation(es[:], sp[:], AF.Exp, scale=ISQ)
                    for d2 in range(2):
                        jt = 2 * pr + d2
                        esd = es[:, d2 * ICH:(d2 + 1) * ICH]
                        if jt >= 4 * ic:
                            nc.vector.tensor_mul(
                                esd, esd, dmask[:, jt - 4 * ic, :])
                    return es

                def pv_pair(pr, es):
                    for d2 in range(2):
                        jt = 2 * pr + d2
                        esd = es[:, d2 * ICH:(d2 + 1) * ICH]
                        nc.tensor.matmul(
                            ot[:], Vn[b][:, jt, h * HD:(h + 1) * HD], esd,
                            start=(jt == 0), stop=(jt == njt - 1))
                        if jt == 0:
                            nc.vector.tensor_copy(acc[:], esd)
                        else:
                            nc.vector.tensor_add(acc[:], acc[:], esd)

                prev = s_pair(0)
                for pr in range(1, npr):
                    cur = s_pair(pr)
                    pv_pair(pr - 1, prev)
                    prev = cur
                pv_pair(npr - 1, prev)
                dbc = d_ps.tile([128, ICH], f32, tag="dbc", name="dbc")
                nc.tensor.matmul(dbc[:], ones_sq_b[:], acc[:])
                lnd = ph2.tile([128, ICH], f32, tag="lnd", name="lnd")
                nc.scalar.activation(lnd[:], dbc[:], AF.Ln)
                bcs = ph2.tile([128, ICH], f32, tag="bcs", name="bcs")
                nc.scalar.activation(bcs[:], lnd[:], AF.Exp, scale=-1.0)
                otn = ph2.tile([128, ICH], bf16, tag="otn", name="otn")
                nc.vector.tensor_mul(otn[:], ot[:], bcs[:])
                nc.scalar.dma_start(
                    ag_in[b][ic // 2][h * HD:(h + 1) * HD,
                                      (ic % 2) * ICH:(ic % 2 + 1) * ICH],
                    otn[:])

            def wo_chunk(b, qc):
                KG = NKT // 2
                ops = [ph3_ps.tile([128, 512], f32, tag="op%d" % oc,
                                   name="op%d" % oc) for oc in range(2)]
                for g in range(2):
                    otf = ph3.tile([128, KG, 512], bf16, tag="otf",
                                   name="otf")
                    nc.sync.dma_start(
                        otf[:],
                        ag_out[b][qc // 2].rearrange("(k p) s -> p k s",
                                                     p=128)
                        [:, g * KG:(g + 1) * KG,
                         (qc % 2) * 512:(qc % 2 + 1) * 512])
                    for oc in range(2):
                        for kk in range(KG):
                            k = g * KG + kk
                            nc.tensor.matmul(
                                ops[oc][:],
                                w_sb["wo"][:, k, oc * 128:(oc + 1) * 128],
                                otf[:, kk, :],
                                start=(k == 0), stop=(k == NKT - 1))
                for oc in range(2):
                    os_ = ph3s.tile([128, 512], f32, tag="os_", name="os_")
                    nc.scalar.copy(os_[:], ops[oc][:])
                    nc.sync.dma_start(
                        out[oc * 128:(oc + 1) * 128,
                            b * S + qc * 512:b * S + (qc + 1) * 512],
                        os_[:])

            def emit_ag(b, hf):
                nc.gpsimd.collective_compute(
                    "AllGather", mybir.AluOpType.bypass,
                    replica_groups=[list(range(NCORES))],
                    ins=[ag_in[b][hf][:, :]], outs=[ag_out[b][hf][:, :]])

            for b in range(B):
                for hf in range(2):
                    for ic in (2 * hf, 2 * hf + 1):
                        for h in range(HL):
                            attn_chunk(b, h, ic)
                    emit_ag(b, hf)
            for qc in range(S // 512):
                wo_chunk(0, qc)
            for qc in range(S // 512):
                wo_chunk(1, qc)

    return nc


def _get_nc(use_w, use_b):
    key = (use_w, use_b)
    if key not in _NC_CACHE:
        _NC_CACHE[key] = _build_nc(use_w, use_b)
    return _NC_CACHE[key]


def _is_causal_mask(mask):
    tril = np.tril(np.ones((S, S), dtype=bool))
    if not np.all(mask[tril] == 0.0):
        return False
    if not np.all(mask[~tril] <= -1e8):
        return False
    return True


def _prep_in_maps(inputs):
    x = np.ascontiguousarray(inputs["x"].astype(np.float32).reshape(BS, D))
    fc = np.ascontiguousarray(inputs["freqs_cos"].astype(np.float32))
    fs = np.ascontiguousarray(inputs["freqs_sin"].astype(np.float32))
    wq, wk, wv, wo = (inputs[k].astype(np.float32)
                      for k in ("wq", "wk", "wv", "wo"))
    qw = inputs["q_ln_w"].astype(np.float32)
    qb = inputs["q_ln_b"].astype(np.float32)
    kw = inputs["k_ln_w"].astype(np.float32)
    kb = inputs["k_ln_b"].astype(np.float32)
    lnwb = np.concatenate([qw[0::2], qw[1::2], kw[0::2], kw[1::2],
                           qb[0::2], qb[1::2], kb[0::2], kb[1::2]])
    lnwb = np.ascontiguousarray(lnwb.reshape(1, 512))
    eye = np.eye(128, dtype=np.float32)
    ones = np.ones((128, 128), dtype=np.float32)
    perm_l = np.concatenate(
        [np.concatenate([np.arange(h * HD, (h + 1) * HD, 2),
                         np.arange(h * HD + 1, (h + 1) * HD, 2)])
         for h in range(HL)])
    in_maps = []
    for c in range(NCORES):
        sl = slice(c * DL, (c + 1) * DL)
        in_maps.append({
            "x": x,
            "wq": np.ascontiguousarray(wq[:, sl][:, perm_l]),
            "wk": np.ascontiguousarray(wk[:, sl][:, perm_l]),
            "wv": np.ascontiguousarray(wv[:, sl]),
            "wo": np.ascontiguousarray(wo[:, sl]),
            "fcos": fc, "fsin": fs,
            "lnwb": lnwb, "eye": eye, "ones": ones,
        })
    use_w = not (np.all(qw == 1.0) and np.all(kw == 1.0))
    use_b = not (np.all(qb == 0.0) and np.all(kb == 0.0))
    return in_maps, use_w, use_b


def _kernel_bass(inputs, trace=False):
    from concourse.bass_utils import run_bass_kernel_spmd
    _install_wait_split()
    if not _is_causal_mask(np.asarray(inputs["mask"], dtype=np.float32)):
        raise ValueError("non-causal mask: bass fast path inapplicable")
    in_maps, use_w, use_b = _prep_in_maps(inputs)
    nc = _get_nc(use_w, use_b)
    res = run_bass_kernel_spmd(nc, in_maps, core_ids=list(range(NCORES)),
                               trace=trace)
    full = np.empty((BS, D), dtype=np.float32)
    for c in range(NCORES):
        full[:, c * DL:(c + 1) * DL] = res.results[c]["out"].T
    o = full.reshape(B, S, D)
    if trace:
        return o, res
    return o


# ---------------------------------------------------------------------------
# Fallbacks (jax pmap tensor-parallel; numpy) — used only if the bass path
# fails (e.g. non-causal mask or no neuron devices).
# ---------------------------------------------------------------------------

def _kernel_jax(inputs):
    import jax
    import jax.numpy as jnp

    devs = jax.devices()[:NCORES]
    assert len(devs) == NCORES

    fc = inputs["freqs_cos"].astype(np.float32)
    fs = inputs["freqs_sin"].astype(np.float32)
    mask = inputs["mask"].astype(np.float32)
    wq, wk, wv, wo = (inputs[k].astype(np.float32)
                      for k in ("wq", "wk", "wv", "wo"))
    qw, qb = (inputs["q_ln_w"].astype(np.float32),
              inputs["q_ln_b"].astype(np.float32))
    kw, kb = (inputs["k_ln_w"].astype(np.float32),
              inputs["k_ln_b"].astype(np.float32))

    wq_s = np.stack([wq[:, c * DL:(c + 1) * DL] for c in range(NCORES)])
    wk_s = np.stack([wk[:, c * DL:(c + 1) * DL] for c in range(NCORES)])
    wv_s = np.stack([wv[:, c * DL:(c + 1) * DL] for c in range(NCORES)])
    wo_s = np.stack([wo[c * DL:(c + 1) * DL, :] for c in range(NCORES)])

    def _ln(t, w, b):
        mu = jnp.mean(t, axis=-1, keepdims=True)
        var = jnp.mean(jnp.square(t - mu), axis=-1, keepdims=True)
        return (t - mu) * jax.lax.rsqrt(var + EPS) * w + b

    def _rope(t, c, s):
        e, o = t[..., 0::2], t[..., 1::2]
        cc = c[None, :, None, :]
        ss = s[None, :, None, :]
        oe = e * cc - o * ss
        oo = e * ss + o * cc
        return jnp.stack([oe, oo], axis=-1).reshape(t.shape)

    def shard_fn(wq_c, wk_c, wv_c, wo_c, x_c, fc_c, fs_c, m_c, qw_c, qb_c,
                 kw_c, kb_c):
        b, s, _ = x_c.shape
        q = (x_c.reshape(b * s, D) @ wq_c).reshape(b, s, HL, HD)
        k = (x_c.reshape(b * s, D) @ wk_c).reshape(b, s, HL, HD)
        v = (x_c.reshape(b * s, D) @ wv_c).reshape(b, s, HL, HD)
        q = _ln(q, qw_c, qb_c)
        k = _ln(k, kw_c, kb_c)
        q = _rope(q, fc_c, fs_c)
        k = _rope(k, fc_c, fs_c)
        scores = jnp.einsum("bqhd,bkhd->bhqk", q, k) * ISQ
        scores = scores + m_c[None, None, :, :]
        probs = jax.nn.softmax(scores, axis=-1)
        o = jnp.einsum("bhqk,bkhd->bqhd", probs, v).reshape(b, s, HL * HD)
        part = o.reshape(b * s, HL * HD) @ wo_c
        return jax.lax.psum(part.reshape(b, s, D), "i")

    pfn = jax.pmap(
        shard_fn, axis_name="i",
        in_axes=(0, 0, 0, 0, None, None, None, None, None, None, None, None),
        devices=devs)
    res = pfn(wq_s, wk_s, wv_s, wo_s, inputs["x"].astype(np.float32), fc, fs,
              mask, qw, qb, kw, kb)
    return np.asarray(res[0], dtype=np.float32)


def _kernel_numpy(inputs):
    x = inputs["x"].astype(np.float32)
    fc, fs = inputs["freqs_cos"], inputs["freqs_sin"]
    mask = inputs["mask"]
    wq, wk, wv, wo = inputs["wq"], inputs["wk"], inputs["wv"], inputs["wo"]
    qw, qb = inputs["q_ln_w"], inputs["q_ln_b"]
    kw, kb = inputs["k_ln_w"], inputs["k_ln_b"]

    def ln(t, w, b):
        mu = t.mean(-1, keepdims=True)
        var = ((t - mu) ** 2).mean(-1, keepdims=True)
        return (t - mu) / np.sqrt(var + EPS) * w + b

    def rope(t):
        e, o = t[..., 0::2], t[..., 1::2]
        c = fc[None, :, None, :]
        s = fs[None, :, None, :]
        o2 = np.empty_like(t)
        o2[..., 0::2] = e * c - o * s
        o2[..., 1::2] = e * s + o * c
        return o2

    b, s, _ = x.shape
    q = (x @ wq).reshape(b, s, H, HD)
    k = (x @ wk).reshape(b, s, H, HD)
    v = (x @ wv).reshape(b, s, H, HD)
    q = rope(ln(q, qw, qb))
    k = rope(ln(k, kw, kb))
    o = np.empty((b, s, H, HD), dtype=np.float32)
    for bi in range(b):
        for h in range(H):
            sc = (q[bi, :, h, :] @ k[bi, :, h, :].T) * ISQ + mask
            sc -= sc.max(-1, keepdims=True)
            p = np.exp(sc)
            p /= p.sum(-1, keepdims=True)
            o[bi, :, h, :] = p @ v[bi, :, h, :]
    return (o.reshape(b, s, D) @ wo).astype(np.float32)


def kernel(**inputs) -> np.ndarray:
    try:
        return _kernel_bass(inputs)
    except Exception:
        try:
            return _kernel_jax(inputs)
        except Exception:
            return _kernel_numpy(inputs)
